# revision 12
# baseline (speedup 1.0000x reference)
# Trainium2 Bass kernel for FJSP actor head (gnn_message_passing).
#
# Math (per batch b):
#   job_emb = ops_emb[b, next_op[b], :]                  [50, 128]  (gather)
#   u_j = job_emb @ W1[:128]   v_m = ma_emb[b] @ W1[128:]
#   h1[j,m] = relu(u_j + v_m + b1)            -> 2000 pairs + 1 noop (dummy)
#   h2 = relu(h1 @ W2 + b2);  logit = h2 @ W3 + b3
#
# Device strategy (pure data parallel over batch, 32 batches/core):
#   * The pairwise broadcast u_j + v_m + b1 is ONE matmul per batch:
#     lhsT = JV (rows: 50 u's, 40 v's, dummy@W1, b1) [106, 128],
#     rhs = S, a constant 0/1 selection matrix [106, 2001] built on host.
#     Column 0 of S selects {dummy@W1, b1} = the noop logit pipeline.
#   * relu1 on DVE/ACT, W2 matmul, relu2(+b2 bias) on ACT/DVE, W3 matmul
#     col-tiled so 4 output chunks land on psum partitions {0,32,64,96},
#     one wide copy to SBUF, one strided DMA per batch row.
#   * b3 is added on host (scalar add during unshard).

import numpy as np
from contextlib import ExitStack

import concourse.bass as bass
import concourse.mybir as mybir
import concourse.tile as tile
from concourse import bacc
from concourse.bass_utils import run_bass_kernel_spmd
from concourse.masks import make_identity

BS, N_OPS, N_JOBS, N_MA, E, H = 256, 2000, 50, 40, 128, 128
NCORES = 8
BPC = BS // NCORES            # 32 batches per core
NPAIR = N_JOBS * N_MA + 1     # 2001 logits per batch (col 0 = noop)
NPAD = 2048                   # padded output row (chunk 3 writes junk past 2001)
PB = 64                       # gather rows reserved per batch (50 real + 14 pad)
NCHUNK = BPC * PB // 128      # 16 gather chunks of 128 rows
# JV partition layout (K = 106)
R_V0 = 64                     # v_m rows 64..103  (u_j rows at 0..49)
R_ZD = 104                    # dummy @ W1 row
R_B1 = 105                    # b1 row
KJV = 106
COLCH = [(0, 512), (512, 512), (1024, 512), (1536, 465)]  # logit col chunks

f32 = mybir.dt.float32


def _build_smat() -> np.ndarray:
    S = np.zeros((KJV, NPAIR), np.float32)
    S[R_B1, :] = 1.0
    S[R_ZD, 0] = 1.0
    for j in range(N_JOBS):
        S[j, 1 + j * N_MA: 1 + (j + 1) * N_MA] = 1.0
    for m in range(N_MA):
        S[R_V0 + m, 1 + m:: N_MA] = 1.0
    return S


def _build_module() -> bass.Bass:
    nc = bacc.Bacc("TRN2", target_bir_lowering=False, debug=False)
    ops = nc.dram_tensor("ops", [BPC * N_OPS, E], f32, kind="ExternalInput")
    ma = nc.dram_tensor("ma", [BPC * N_MA, E], f32, kind="ExternalInput")
    idx = nc.dram_tensor("idx", [128, NCHUNK], mybir.dt.int32, kind="ExternalInput")
    smat = nc.dram_tensor("smat", [KJV, NPAIR], f32, kind="ExternalInput")
    w1 = nc.dram_tensor("w1", [2 * E, H], f32, kind="ExternalInput")
    w2 = nc.dram_tensor("w2", [H, H], f32, kind="ExternalInput")
    w3 = nc.dram_tensor("w3", [H, 1], f32, kind="ExternalInput")
    b1v = nc.dram_tensor("b1v", [H], f32, kind="ExternalInput")
    b2v = nc.dram_tensor("b2v", [H], f32, kind="ExternalInput")
    dvec = nc.dram_tensor("dvec", [2 * E], f32, kind="ExternalInput")
    out = nc.dram_tensor("out", [BPC, NPAD], f32, kind="ExternalOutput")

    Relu = mybir.ActivationFunctionType.Relu

    with tile.TileContext(nc) as tc, ExitStack() as ctx:
        singles = ctx.enter_context(tc.tile_pool(name="singles", bufs=1))

        ident = singles.tile([128, 128], f32)
        make_identity(nc, ident[:])

        wj_s = singles.tile([128, H], f32)
        nc.sync.dma_start(out=wj_s[:], in_=w1[0:E, :])
        wm_s = singles.tile([128, H], f32)
        nc.sync.dma_start(out=wm_s[:], in_=w1[E:2 * E, :])
        w2_s = singles.tile([128, H], f32)
        nc.sync.dma_start(out=w2_s[:], in_=w2[:])
        w3_s = singles.tile([128, 1], f32)
        nc.sync.dma_start(out=w3_s[:], in_=w3[:])
        b2_s = singles.tile([128, 1], f32)
        nc.sync.dma_start(out=b2_s[:], in_=b2v[:].rearrange("(p o) -> p o", o=1))
        smat_s = singles.tile([KJV, NPAIR], f32)
        nc.sync.dma_start(out=smat_s[:], in_=smat[:])
        idx_s = singles.tile([128, NCHUNK], mybir.dt.int32)
        nc.sync.dma_start(out=idx_s[:], in_=idx[:])
        # dummy halves as two columns (partition-strided load, one-time)
        dcols = singles.tile([128, 2], f32)
        nc.sync.dma_start(out=dcols[:, 0:1], in_=dvec[0:E].rearrange("(p o) -> p o", o=1))
        nc.sync.dma_start(out=dcols[:, 1:2], in_=dvec[E:2 * E].rearrange("(p o) -> p o", o=1))
        # zdb1: row 0 = dummy@W1 (filled below), row 1 = b1
        zdb1 = singles.tile([2, 128], f32)
        nc.sync.dma_start(out=zdb1[1:2, :], in_=b1v[:].rearrange("(o e) -> o e", o=1))

        # per-batch JV tiles (stationary operand of the S-matmul)
        jv_pool = ctx.enter_context(tc.tile_pool(name="jvp", bufs=BPC))
        jv = [jv_pool.tile([128, 128], f32, tag="jv", name=f"jv{b}")
              for b in range(BPC)]
        for b in range(BPC):
            nc.gpsimd.memset(jv[b][:], 0.0)

        setup_ps = ctx.enter_context(tc.tile_pool(name="sps", bufs=2, space="PSUM"))

        # dummy @ W1 (once): [1,128] = dcols[:,0].T @ Wj + dcols[:,1].T @ Wm
        pd = setup_ps.tile([1, 128], f32, tag="sps")
        nc.tensor.matmul(out=pd[:], lhsT=dcols[:, 0:1], rhs=wj_s[:], start=True, stop=False)
        nc.tensor.matmul(out=pd[:], lhsT=dcols[:, 1:2], rhs=wm_s[:], start=False, stop=True)
        nc.vector.tensor_copy(out=zdb1[0:1, :], in_=pd[:])

        gather_pool = ctx.enter_context(tc.tile_pool(name="gath", bufs=3))
        jt_pool = ctx.enter_context(tc.tile_pool(name="jt", bufs=3))
        mt_pool = ctx.enter_context(tc.tile_pool(name="mt", bufs=3))
        h1_ps = ctx.enter_context(tc.tile_pool(name="h1ps", bufs=2, space="PSUM"))
        h2_ps = ctx.enter_context(tc.tile_pool(name="h2ps", bufs=2, space="PSUM"))
        lg_ps = ctx.enter_context(tc.tile_pool(name="lgps", bufs=2, space="PSUM"))
        a_pool = ctx.enter_context(tc.tile_pool(name="ap", bufs=3))
        h2s_pool = ctx.enter_context(tc.tile_pool(name="h2s", bufs=3))
        st_pool = ctx.enter_context(tc.tile_pool(name="st", bufs=2))

        for c in range(NCHUNK):
            # ---- gather 128 job rows (2 batches worth, 64-row stride) ----
            grows = gather_pool.tile([128, E], f32, tag="grows")
            nc.gpsimd.indirect_dma_start(
                out=grows[:], out_offset=None, in_=ops[:],
                in_offset=bass.IndirectOffsetOnAxis(ap=idx_s[:, c:c + 1], axis=0),
            )
            tpj = setup_ps.tile([128, 128], f32, tag="sps")
            nc.tensor.transpose(out=tpj[:], in_=grows[:], identity=ident[:])
            jTc = jt_pool.tile([128, 128], f32, tag="jt")
            nc.scalar.copy(out=jTc[:], in_=tpj[:])

            # ---- ma rows for the same 2 batches ----
            # rows 80:128 stay junk; transposed junk cols are never read
            mrows = gather_pool.tile([128, E], f32, tag="mrows")
            b0 = 2 * c
            nc.sync.dma_start(out=mrows[0:2 * N_MA, :],
                              in_=ma[b0 * N_MA:(b0 + 2) * N_MA, :])
            tpm = setup_ps.tile([128, 128], f32, tag="sps")
            nc.tensor.transpose(out=tpm[:], in_=mrows[:], identity=ident[:])
            mTc = mt_pool.tile([128, 128], f32, tag="mt")
            nc.vector.tensor_copy(out=mTc[:], in_=tpm[:])

            for sub in range(2):
                b = 2 * c + sub
                # ---- projections into JV[b] ----
                pj = setup_ps.tile([KJV, 128], f32, tag="sps")
                nc.tensor.matmul(out=pj[0:N_JOBS, :],
                                 lhsT=jTc[:, sub * PB: sub * PB + N_JOBS],
                                 rhs=wj_s[:], start=True, stop=True)
                nc.tensor.matmul(out=pj[R_V0:R_V0 + N_MA, :],
                                 lhsT=mTc[:, sub * N_MA: (sub + 1) * N_MA],
                                 rhs=wm_s[:], start=True, stop=True)
                nc.scalar.copy(out=jv[b][0:N_JOBS, :], in_=pj[0:N_JOBS, :])
                nc.vector.tensor_copy(out=jv[b][R_V0:R_V0 + N_MA, :],
                                      in_=pj[R_V0:R_V0 + N_MA, :])
                nc.sync.dma_start(out=jv[b][R_ZD:R_B1 + 1, :], in_=zdb1[:])

                # ---- main pipeline for batch b ----
                lgp = lg_ps.tile([128, 512], f32, tag="lg")
                for ci, (c0, cw) in enumerate(COLCH):
                    h1p = h1_ps.tile([128, 512], f32, tag="h1p")
                    nc.tensor.matmul(out=h1p[:, :cw], lhsT=jv[b][0:KJV, :],
                                     rhs=smat_s[:, c0:c0 + cw], start=True, stop=True)
                    A = a_pool.tile([128, 512], f32, tag="A")
                    if ci % 2 == 0:
                        nc.vector.tensor_scalar_max(out=A[:, :cw], in0=h1p[:, :cw],
                                                    scalar1=0.0)
                    else:
                        nc.scalar.activation(out=A[:, :cw], in_=h1p[:, :cw], func=Relu)
                    h2p = h2_ps.tile([128, 512], f32, tag="h2p")
                    nc.tensor.matmul(out=h2p[:, :cw], lhsT=w2_s[:], rhs=A[:, :cw],
                                     start=True, stop=True)
                    H2 = h2s_pool.tile([128, 512], f32, tag="H2")
                    if ci % 2 == 0:
                        nc.scalar.activation(out=H2[:, :cw], in_=h2p[:, :cw],
                                             func=Relu, bias=b2_s[:, 0:1])
                    else:
                        nc.vector.tensor_scalar(out=H2[:, :cw], in0=h2p[:, :cw],
                                                scalar1=b2_s[:, 0:1], scalar2=0.0,
                                                op0=mybir.AluOpType.add,
                                                op1=mybir.AluOpType.max)
                    # logits chunk -> psum partition 32*ci
                    nc.tensor.matmul(out=lgp[32 * ci:32 * ci + 1, :cw], lhsT=w3_s[:],
                                     rhs=H2[:, :cw], start=True, stop=True,
                                     tile_position=(0, 32 * ci))
                # one wide copy (junk partitions included; only rows 0/32/64/96 real)
                stg = st_pool.tile([128, 512], f32, tag="st")
                if b % 2 == 0:
                    nc.scalar.copy(out=stg[0:97, :], in_=lgp[0:97, :])
                else:
                    nc.vector.tensor_copy(out=stg[0:97, :], in_=lgp[0:97, :])
                stg4 = stg[:].rearrange("(a b) f -> a b f", b=32)[:, 0:1, :]
                nc.sync.dma_start(
                    out=out[b:b + 1, :].rearrange("o (a f) -> o a f", a=4),
                    in_=stg4)

    nc.finalize()
    return nc


_CACHE: dict = {}


def _get_module() -> bass.Bass:
    if "nc" not in _CACHE:
        _CACHE["nc"] = _build_module()
    return _CACHE["nc"]


def _make_in_maps(inputs):
    ops_emb = np.ascontiguousarray(np.asarray(inputs["ops_emb"], dtype=np.float32))
    ma_emb = np.ascontiguousarray(np.asarray(inputs["ma_emb"], dtype=np.float32))
    next_op = np.asarray(inputs["next_op"])
    action_mask = np.asarray(inputs["action_mask"])
    dummy = np.asarray(inputs["dummy"], dtype=np.float32)
    W1 = np.ascontiguousarray(np.asarray(inputs["W1"], dtype=np.float32))
    b1 = np.ascontiguousarray(np.asarray(inputs["b1"], dtype=np.float32))
    W2 = np.ascontiguousarray(np.asarray(inputs["W2"], dtype=np.float32))
    b2 = np.ascontiguousarray(np.asarray(inputs["b2"], dtype=np.float32))
    W3 = np.ascontiguousarray(np.asarray(inputs["W3"], dtype=np.float32))
    smat = _build_smat()

    in_maps = []
    for core in range(NCORES):
        bsl = slice(core * BPC, (core + 1) * BPC)
        no = np.asarray(next_op[bsl], dtype=np.int64)          # [BPC, 50]
        gidx = np.zeros((BPC, PB), np.int64)
        gidx[:, :N_JOBS] = no + (np.arange(BPC, dtype=np.int64)[:, None] * N_OPS)
        idx2d = np.ascontiguousarray(
            gidx.reshape(NCHUNK, 128).T.astype(np.int32))      # [128, NCHUNK]
        in_maps.append({
            "ops": ops_emb[bsl].reshape(BPC * N_OPS, E),
            "ma": ma_emb[bsl].reshape(BPC * N_MA, E),
            "idx": idx2d,
            "smat": smat,
            "w1": W1, "w2": W2, "w3": W3,
            "b1v": b1, "b2v": b2, "dvec": dummy,
        })
    return in_maps


def _run(inputs, trace=False, **kw):
    action_mask = np.asarray(inputs["action_mask"])
    b3 = np.asarray(inputs["b3"], dtype=np.float32)
    nc = _get_module()
    in_maps = _make_in_maps(inputs)
    res = run_bass_kernel_spmd(nc, in_maps, core_ids=list(range(NCORES)),
                               trace=trace, **kw)
    logits = np.concatenate([r["out"][:, :NPAIR] for r in res.results], axis=0)
    logits = (logits + b3.reshape(-1)[0]).astype(np.float32)
    return (logits, action_mask), res


def kernel(**inputs):
    out, _ = _run(inputs)
    return out


# revision 17
# speedup vs baseline: 2.1033x; 2.1033x over previous
# Trainium2 Bass kernel for FJSP actor head (gnn_message_passing).
#
# Math (per batch b):
#   job_emb = ops_emb[b, next_op[b], :]                  [50, 128]  (gather)
#   u_j = job_emb @ W1[:128]   v_m = ma_emb[b] @ W1[128:]
#   h1[j,m] = relu(u_j + v_m + b1)            -> 2000 pairs + 1 noop (dummy)
#   h2 = relu(h1 @ W2 + b2);  logit = h2 @ W3 + b3
#
# Device strategy (pure data parallel over batch, 32 batches/core):
#   * The pairwise broadcast u_j + v_m + b1 is ONE matmul per batch:
#     lhsT = JV (rows: 50 u's, 40 v's, dummy@W1, b1) [106, 128],
#     rhs = S, a constant 0/1 selection matrix [106, 2001] built on host.
#     Column 0 of S selects {dummy@W1, b1} = the noop logit pipeline.
#   * relu1 on DVE/ACT, W2 matmul, relu2(+b2 bias) on ACT/DVE, W3 matmul
#     col-tiled so 4 output chunks land on psum partitions {0,32,64,96},
#     one wide copy to SBUF, one strided DMA per batch row.
#   * b3 is added on host (scalar add during unshard).

import numpy as np
from contextlib import ExitStack

import concourse.bass as bass
import concourse.mybir as mybir
import concourse.tile as tile
from concourse import bacc
from concourse.bass_utils import run_bass_kernel_spmd
from concourse.masks import make_identity

BS, N_OPS, N_JOBS, N_MA, E, H = 256, 2000, 50, 40, 128, 128
NCORES = 8
BPC = BS // NCORES            # 32 batches per core
NPAIR = N_JOBS * N_MA + 1     # 2001 logits per batch (col 0 = noop)
NPAD = 2048                   # padded output row (chunk 3 writes junk past 2001)
PB = 64                       # gather rows reserved per batch (50 real + 14 pad)
NCHUNK = BPC * PB // 128      # 16 gather chunks of 128 rows
# JV partition layout (K = 106)
R_V0 = 64                     # v_m rows 64..103  (u_j rows at 0..49)
R_ZD = 104                    # dummy @ W1 row
R_B1 = 105                    # b1 row
KJV = 106
COLCH = [(0, 512), (512, 512), (1024, 512), (1536, 512)]  # logit col chunks

f32 = mybir.dt.float32
f32r = mybir.dt.float32r
bf16 = mybir.dt.bfloat16


def _build_smat() -> np.ndarray:
    S = np.zeros((KJV, NPAD), np.float32)
    S[R_B1, :] = 1.0
    S[R_ZD, 0] = 1.0
    for j in range(N_JOBS):
        S[j, 1 + j * N_MA: 1 + (j + 1) * N_MA] = 1.0
    for m in range(N_MA):
        S[R_V0 + m, 1 + m:: N_MA] = 1.0
    return S


def _build_module() -> bass.Bass:
    nc = bacc.Bacc("TRN2", target_bir_lowering=False, debug=False)
    ops = nc.dram_tensor("ops", [BPC * N_OPS, E], f32, kind="ExternalInput")
    ma = nc.dram_tensor("ma", [BPC * N_MA, E], f32, kind="ExternalInput")
    idx = nc.dram_tensor("idx", [128, NCHUNK], mybir.dt.int32, kind="ExternalInput")
    smat = nc.dram_tensor("smat", [KJV, NPAD], f32r, kind="ExternalInput")
    w1 = nc.dram_tensor("w1", [2 * E, H], f32, kind="ExternalInput")
    w2 = nc.dram_tensor("w2", [H, H], f32r, kind="ExternalInput")
    w3 = nc.dram_tensor("w3", [H, 1], bf16, kind="ExternalInput")
    b1v = nc.dram_tensor("b1v", [H], f32r, kind="ExternalInput")
    b2v = nc.dram_tensor("b2v", [H], f32, kind="ExternalInput")
    dvec = nc.dram_tensor("dvec", [2 * E], f32, kind="ExternalInput")
    out = nc.dram_tensor("out", [BPC, NPAD], f32, kind="ExternalOutput")

    Relu = mybir.ActivationFunctionType.Relu

    with tile.TileContext(nc) as tc, ExitStack() as ctx:
        singles = ctx.enter_context(tc.tile_pool(name="singles", bufs=1))

        ident = singles.tile([128, 128], f32)
        make_identity(nc, ident[:])

        wj_s = singles.tile([128, H], f32)
        nc.sync.dma_start(out=wj_s[:], in_=w1[0:E, :])
        wm_s = singles.tile([128, H], f32)
        nc.sync.dma_start(out=wm_s[:], in_=w1[E:2 * E, :])
        w2_s = singles.tile([128, H], f32r)
        nc.sync.dma_start(out=w2_s[:], in_=w2[:])
        w3_s = singles.tile([128, 1], bf16)
        nc.sync.dma_start(out=w3_s[:], in_=w3[:])
        b2_s = singles.tile([128, 1], f32)
        nc.sync.dma_start(out=b2_s[:], in_=b2v[:].rearrange("(p o) -> p o", o=1))
        smat_s = singles.tile([KJV, NPAD], f32r)
        nc.sync.dma_start(out=smat_s[:], in_=smat[:])
        idx_s = singles.tile([128, NCHUNK], mybir.dt.int32)
        nc.sync.dma_start(out=idx_s[:], in_=idx[:])
        # dummy halves as two columns (partition-strided load, one-time)
        dcols = singles.tile([128, 2], f32)
        nc.sync.dma_start(out=dcols[:, 0:1], in_=dvec[0:E].rearrange("(p o) -> p o", o=1))
        nc.sync.dma_start(out=dcols[:, 1:2], in_=dvec[E:2 * E].rearrange("(p o) -> p o", o=1))
        # zdb1: row 0 = dummy@W1 (filled below), row 1 = b1
        zdb1 = singles.tile([2, 128], f32r)
        nc.sync.dma_start(out=zdb1[1:2, :], in_=b1v[:].rearrange("(o e) -> o e", o=1))

        # per-batch JV tiles (stationary operand of the S-matmul)
        jv_pool = ctx.enter_context(tc.tile_pool(name="jvp", bufs=BPC))
        jv = [jv_pool.tile([128, 128], f32r, tag="jv", name=f"jv{b}")
              for b in range(BPC)]
        for b in range(BPC):
            nc.gpsimd.memset(jv[b][:].bitcast(mybir.dt.uint32), 0)

        setup_ps = ctx.enter_context(tc.tile_pool(name="sps", bufs=2, space="PSUM"))

        # dummy @ W1 (once): [1,128] = dcols[:,0].T @ Wj + dcols[:,1].T @ Wm
        pd = setup_ps.tile([1, 128], f32, tag="sps")
        nc.tensor.matmul(out=pd[:], lhsT=dcols[:, 0:1], rhs=wj_s[:], start=True, stop=False)
        nc.tensor.matmul(out=pd[:], lhsT=dcols[:, 1:2], rhs=wm_s[:], start=False, stop=True)
        nc.vector.tensor_copy(out=zdb1[0:1, :], in_=pd[:])

        gather_pool = ctx.enter_context(tc.tile_pool(name="gath", bufs=3))
        jt_pool = ctx.enter_context(tc.tile_pool(name="jt", bufs=3))
        mt_pool = ctx.enter_context(tc.tile_pool(name="mt", bufs=3))
        h1_ps = ctx.enter_context(tc.tile_pool(name="h1ps", bufs=2, space="PSUM"))
        h2_ps = ctx.enter_context(tc.tile_pool(name="h2ps", bufs=2, space="PSUM"))
        lg_ps = ctx.enter_context(tc.tile_pool(name="lgps", bufs=2, space="PSUM"))
        a_pool = ctx.enter_context(tc.tile_pool(name="ap", bufs=3))
        h2s_pool = ctx.enter_context(tc.tile_pool(name="h2s", bufs=3))
        st_pool = ctx.enter_context(tc.tile_pool(name="st", bufs=2))

        for c in range(NCHUNK):
            # ---- gather 128 job rows (2 batches worth, 64-row stride) ----
            grows = gather_pool.tile([128, E], f32, tag="grows")
            nc.gpsimd.indirect_dma_start(
                out=grows[:], out_offset=None, in_=ops[:],
                in_offset=bass.IndirectOffsetOnAxis(ap=idx_s[:, c:c + 1], axis=0),
            )
            tpj = setup_ps.tile([128, 128], f32, tag="sps")
            nc.tensor.transpose(out=tpj[:], in_=grows[:], identity=ident[:])
            jTc = jt_pool.tile([128, 128], f32, tag="jt")
            nc.scalar.copy(out=jTc[:], in_=tpj[:])

            # ---- ma rows for the same 2 batches ----
            # rows 80:128 stay junk; transposed junk cols are never read
            mrows = gather_pool.tile([128, E], f32, tag="mrows")
            b0 = 2 * c
            nc.sync.dma_start(out=mrows[0:2 * N_MA, :],
                              in_=ma[b0 * N_MA:(b0 + 2) * N_MA, :])
            tpm = setup_ps.tile([128, 128], f32, tag="sps")
            nc.tensor.transpose(out=tpm[:], in_=mrows[:], identity=ident[:])
            mTc = mt_pool.tile([128, 128], f32, tag="mt")
            nc.vector.tensor_copy(out=mTc[:], in_=tpm[:])

            for sub in range(2):
                b = 2 * c + sub
                # ---- projections into JV[b] ----
                pj = setup_ps.tile([KJV, 128], f32, tag="sps")
                nc.tensor.matmul(out=pj[0:N_JOBS, :],
                                 lhsT=jTc[:, sub * PB: sub * PB + N_JOBS],
                                 rhs=wj_s[:], start=True, stop=True)
                nc.tensor.matmul(out=pj[R_V0:R_V0 + N_MA, :],
                                 lhsT=mTc[:, sub * N_MA: (sub + 1) * N_MA],
                                 rhs=wm_s[:], start=True, stop=True)
                nc.scalar.copy(out=jv[b][0:N_JOBS, :], in_=pj[0:N_JOBS, :])
                nc.vector.tensor_copy(out=jv[b][R_V0:R_V0 + N_MA, :],
                                      in_=pj[R_V0:R_V0 + N_MA, :])
                nc.sync.dma_start(out=jv[b][R_ZD:R_B1 + 1, :], in_=zdb1[:])

                # ---- main pipeline for batch b ----
                lgp = lg_ps.tile([128, 512], f32, tag="lg")
                for ci, (c0, cw) in enumerate(COLCH):
                    h1p = h1_ps.tile([128, 512], f32, tag="h1p")
                    nc.tensor.matmul(
                        out=h1p[:, :cw], lhsT=jv[b][0:KJV, :],
                        rhs=smat_s[:, c0:c0 + cw],
                        start=True, stop=True)
                    A = a_pool.tile([128, 512], f32r, tag="A")
                    if ci % 2 == 0:
                        nc.vector.tensor_scalar_max(out=A[:, :cw], in0=h1p[:, :cw],
                                                    scalar1=0.0)
                    else:
                        nc.scalar.activation(out=A[:, :cw], in_=h1p[:, :cw], func=Relu)
                    h2p = h2_ps.tile([128, 512], f32, tag="h2p")
                    nc.tensor.matmul(out=h2p[:, :cw], lhsT=w2_s[:],
                                     rhs=A[:, :cw],
                                     start=True, stop=True)
                    H2 = h2s_pool.tile([128, 512], bf16, tag="H2")
                    if ci % 2 == 0:
                        nc.scalar.activation(out=H2[:, :cw], in_=h2p[:, :cw],
                                             func=Relu, bias=b2_s[:, 0:1])
                    else:
                        nc.vector.tensor_scalar(out=H2[:, :cw], in0=h2p[:, :cw],
                                                scalar1=b2_s[:, 0:1], scalar2=0.0,
                                                op0=mybir.AluOpType.add,
                                                op1=mybir.AluOpType.max)
                    # logits chunk -> psum partition 32*ci
                    nc.tensor.matmul(out=lgp[32 * ci:32 * ci + 1, :cw],
                                     lhsT=w3_s[:],
                                     rhs=H2[:, :cw],
                                     start=True, stop=True,
                                     tile_position=(0, 32 * ci))
                # one wide copy (junk partitions included; only rows 0/32/64/96 real)
                stg = st_pool.tile([128, 512], f32, tag="st")
                if b % 2 == 0:
                    nc.scalar.copy(out=stg[0:97, :], in_=lgp[0:97, :])
                else:
                    nc.vector.tensor_copy(out=stg[0:97, :], in_=lgp[0:97, :])
                stg4 = stg[:].rearrange("(a b) f -> a b f", b=32)[:, 0:1, :]
                nc.sync.dma_start(
                    out=out[b:b + 1, :].rearrange("o (a f) -> o a f", a=4),
                    in_=stg4)

    nc.finalize()
    return nc


_CACHE: dict = {}


def _get_module() -> bass.Bass:
    if "nc" not in _CACHE:
        _CACHE["nc"] = _build_module()
    return _CACHE["nc"]


def _make_in_maps(inputs):
    ops_emb = np.ascontiguousarray(np.asarray(inputs["ops_emb"], dtype=np.float32))
    ma_emb = np.ascontiguousarray(np.asarray(inputs["ma_emb"], dtype=np.float32))
    next_op = np.asarray(inputs["next_op"])
    action_mask = np.asarray(inputs["action_mask"])
    dummy = np.asarray(inputs["dummy"], dtype=np.float32)
    W1 = np.ascontiguousarray(np.asarray(inputs["W1"], dtype=np.float32))
    b1 = np.ascontiguousarray(np.asarray(inputs["b1"], dtype=np.float32))
    W2 = np.ascontiguousarray(np.asarray(inputs["W2"], dtype=np.float32))
    b2 = np.ascontiguousarray(np.asarray(inputs["b2"], dtype=np.float32))
    W3 = np.ascontiguousarray(np.asarray(inputs["W3"], dtype=np.float32))
    import ml_dtypes
    W3_bf = W3.astype(ml_dtypes.bfloat16)
    smat = _build_smat()

    in_maps = []
    for core in range(NCORES):
        bsl = slice(core * BPC, (core + 1) * BPC)
        no = np.asarray(next_op[bsl], dtype=np.int64)          # [BPC, 50]
        gidx = np.zeros((BPC, PB), np.int64)
        gidx[:, :N_JOBS] = no + (np.arange(BPC, dtype=np.int64)[:, None] * N_OPS)
        idx2d = np.ascontiguousarray(
            gidx.reshape(NCHUNK, 128).T.astype(np.int32))      # [128, NCHUNK]
        in_maps.append({
            "ops": ops_emb[bsl].reshape(BPC * N_OPS, E),
            "ma": ma_emb[bsl].reshape(BPC * N_MA, E),
            "idx": idx2d,
            "smat": smat,
            "w1": W1, "w2": W2, "w3": W3_bf,
            "b1v": b1, "b2v": b2, "dvec": dummy,
        })
    return in_maps


def _run(inputs, trace=False, **kw):
    action_mask = np.asarray(inputs["action_mask"])
    b3 = np.asarray(inputs["b3"], dtype=np.float32)
    nc = _get_module()
    in_maps = _make_in_maps(inputs)
    res = run_bass_kernel_spmd(nc, in_maps, core_ids=list(range(NCORES)),
                               trace=trace, **kw)
    logits = np.concatenate([r["out"][:, :NPAIR] for r in res.results], axis=0)
    logits = (logits + b3.reshape(-1)[0]).astype(np.float32)
    return (logits, action_mask), res


def kernel(**inputs):
    out, _ = _run(inputs)
    return out


# revision 18
# speedup vs baseline: 2.2290x; 1.0598x over previous
# Trainium2 Bass kernel for FJSP actor head (gnn_message_passing).
#
# Math (per batch b):
#   job_emb = ops_emb[b, next_op[b], :]                  [50, 128]  (gather)
#   u_j = job_emb @ W1[:128]   v_m = ma_emb[b] @ W1[128:]
#   h1[j,m] = relu(u_j + v_m + b1)            -> 2000 pairs + 1 noop (dummy)
#   h2 = relu(h1 @ W2 + b2);  logit = h2 @ W3 + b3
#
# Device strategy (pure data parallel over batch, 32 batches/core):
#   * The pairwise broadcast u_j + v_m + b1 is ONE matmul per batch:
#     lhsT = JV (rows: 50 u's, 40 v's, dummy@W1, b1) [106, 128],
#     rhs = S, a constant 0/1 selection matrix built on host.
#     Column 0 of S selects {dummy@W1, b1} = the noop logit pipeline.
#   * relu1/relu2 alternate DVE and ACT; W3 matmul col-tiled so the 4
#     chunks land on psum partitions {0,32,64,96}; one wide copy to SBUF
#     and one strided DMA per batch row.  b3 is added on host.
#   * Matmul stages run bf16 (psum accumulate fp32); projections fp32.

import numpy as np
from contextlib import ExitStack

import concourse.bass as bass
import concourse.mybir as mybir
import concourse.tile as tile
from concourse import bacc
from concourse.bass_utils import run_bass_kernel_spmd
from concourse.masks import make_identity

BS, N_OPS, N_JOBS, N_MA, E, H = 256, 2000, 50, 40, 128, 128
NCORES = 8
BPC = BS // NCORES            # 32 batches per core
NPAIR = N_JOBS * N_MA + 1     # 2001 logits per batch (col 0 = noop)
NPAD = 2048                   # padded logit row (cols 2001:2048 are junk)
PB = 64                       # gather rows reserved per batch (50 real + 14 pad)
NCHUNK = BPC * PB // 128      # 16 gather chunks of 128 rows
# JV partition layout (K = 106)
R_V0 = 64                     # v_m rows 64..103  (u_j rows at 0..49)
R_ZD = 104                    # dummy @ W1 row
R_B1 = 105                    # b1 row
KJV = 106
COLCH = [(0, 512), (512, 512), (1024, 512), (1536, 512)]

f32 = mybir.dt.float32
bf16 = mybir.dt.bfloat16
MMDT = bf16                   # dtype of the big matmul stages


def _build_smat() -> np.ndarray:
    S = np.zeros((KJV, NPAD), np.float32)
    S[R_B1, :NPAIR] = 1.0
    S[R_ZD, 0] = 1.0
    for j in range(N_JOBS):
        S[j, 1 + j * N_MA: 1 + (j + 1) * N_MA] = 1.0
    for m in range(N_MA):
        S[R_V0 + m, 1 + m: NPAIR: N_MA] = 1.0
    return S


def _build_module() -> bass.Bass:
    nc = bacc.Bacc("TRN2", target_bir_lowering=False, debug=False)
    ops = nc.dram_tensor("ops", [BPC * N_OPS, E], f32, kind="ExternalInput")
    ma = nc.dram_tensor("ma", [BPC * N_MA, E], f32, kind="ExternalInput")
    idx = nc.dram_tensor("idx", [128, NCHUNK], mybir.dt.int32, kind="ExternalInput")
    smat = nc.dram_tensor("smat", [KJV, NPAD], MMDT, kind="ExternalInput")
    w1 = nc.dram_tensor("w1", [2 * E, H], f32, kind="ExternalInput")
    w2 = nc.dram_tensor("w2", [H, H], MMDT, kind="ExternalInput")
    w3 = nc.dram_tensor("w3", [H, 1], MMDT, kind="ExternalInput")
    b1v = nc.dram_tensor("b1v", [H], MMDT, kind="ExternalInput")
    b2v = nc.dram_tensor("b2v", [H], f32, kind="ExternalInput")
    dvec = nc.dram_tensor("dvec", [2 * E], f32, kind="ExternalInput")
    out = nc.dram_tensor("out", [BPC, NPAD], f32, kind="ExternalOutput")

    Relu = mybir.ActivationFunctionType.Relu

    with tile.TileContext(nc) as tc, ExitStack() as ctx:
        singles = ctx.enter_context(tc.tile_pool(name="singles", bufs=1))

        # ---- input loads, ordered so the gather chain starts ASAP ----
        idx_s = singles.tile([128, NCHUNK], mybir.dt.int32)
        nc.sync.dma_start(out=idx_s[:], in_=idx[:])

        ident = singles.tile([128, 128], f32)
        make_identity(nc, ident[:])

        # all 16 indirect gathers up front on the gpsimd queue
        grows_pool = ctx.enter_context(tc.tile_pool(name="growsp", bufs=NCHUNK))
        grows = [grows_pool.tile([128, E], f32, tag="grows", name=f"grows{c}")
                 for c in range(NCHUNK)]
        for c in range(NCHUNK):
            nc.gpsimd.indirect_dma_start(
                out=grows[c][:], out_offset=None, in_=ops[:],
                in_offset=bass.IndirectOffsetOnAxis(ap=idx_s[:, c:c + 1], axis=0),
            )

        # ma loads on the sync queue (rows 80:128 junk, never read downstream)
        mrows_pool = ctx.enter_context(tc.tile_pool(name="mrowsp", bufs=NCHUNK))
        mrows = [mrows_pool.tile([128, E], f32, tag="mrows", name=f"mrows{c}")
                 for c in range(NCHUNK)]
        for c in range(NCHUNK):
            nc.sync.dma_start(out=mrows[c][0:2 * N_MA, :],
                              in_=ma[2 * c * N_MA:(2 * c + 2) * N_MA, :])

        wj_s = singles.tile([128, H], f32)
        nc.sync.dma_start(out=wj_s[:], in_=w1[0:E, :])
        wm_s = singles.tile([128, H], f32)
        nc.sync.dma_start(out=wm_s[:], in_=w1[E:2 * E, :])
        w2_s = singles.tile([128, H], MMDT)
        nc.sync.dma_start(out=w2_s[:], in_=w2[:])
        w3_s = singles.tile([128, 1], MMDT)
        nc.sync.dma_start(out=w3_s[:], in_=w3[:])
        smat_s = singles.tile([KJV, NPAD], MMDT)
        nc.sync.dma_start(out=smat_s[:], in_=smat[:])

        # small partition-strided loads on the scalar HWDGE ring
        b2_s = singles.tile([128, 1], f32)
        nc.scalar.dma_start(out=b2_s[:], in_=b2v[:].rearrange("(p o) -> p o", o=1))
        dcols = singles.tile([128, 2], f32)
        nc.scalar.dma_start(out=dcols[:, 0:1],
                            in_=dvec[0:E].rearrange("(p o) -> p o", o=1))
        nc.scalar.dma_start(out=dcols[:, 1:2],
                            in_=dvec[E:2 * E].rearrange("(p o) -> p o", o=1))
        # zdb1: row 0 = dummy@W1 (filled below), row 1 = b1
        zdb1 = singles.tile([2, 128], MMDT)
        nc.scalar.dma_start(out=zdb1[1:2, :],
                            in_=b1v[:].rearrange("(o e) -> o e", o=1))

        # per-batch JV tiles; rows 50:64 must be real zeros (S rows are 0 there)
        jv_pool = ctx.enter_context(tc.tile_pool(name="jvp", bufs=BPC))
        jv = [jv_pool.tile([128, 128], MMDT, tag="jv", name=f"jv{b}")
              for b in range(BPC)]
        for b in range(BPC):
            if b % 2 == 0:
                nc.vector.memset(jv[b][:].bitcast(mybir.dt.uint16), 0)
            else:
                nc.scalar.memzero(jv[b][:])

        setup_ps = ctx.enter_context(tc.tile_pool(name="sps", bufs=2, space="PSUM"))

        # dummy @ W1 (once): [1,128] = dcols[:,0].T @ Wj + dcols[:,1].T @ Wm
        pd = setup_ps.tile([1, 128], f32, tag="sps")
        nc.tensor.matmul(out=pd[:], lhsT=dcols[:, 0:1], rhs=wj_s[:],
                         start=True, stop=False)
        nc.tensor.matmul(out=pd[:], lhsT=dcols[:, 1:2], rhs=wm_s[:],
                         start=False, stop=True)
        nc.vector.tensor_copy(out=zdb1[0:1, :], in_=pd[:])

        jt_pool = ctx.enter_context(tc.tile_pool(name="jt", bufs=3))
        mt_pool = ctx.enter_context(tc.tile_pool(name="mt", bufs=3))
        h1_ps = ctx.enter_context(tc.tile_pool(name="h1ps", bufs=2, space="PSUM"))
        h2_ps = ctx.enter_context(tc.tile_pool(name="h2ps", bufs=2, space="PSUM"))
        lg_ps = ctx.enter_context(tc.tile_pool(name="lgps", bufs=2, space="PSUM"))
        a_pool = ctx.enter_context(tc.tile_pool(name="ap", bufs=3))
        h2s_pool = ctx.enter_context(tc.tile_pool(name="h2s", bufs=3))
        st_pool = ctx.enter_context(tc.tile_pool(name="st", bufs=2))

        for c in range(NCHUNK):
            tpj = setup_ps.tile([128, 128], f32, tag="sps")
            nc.tensor.transpose(out=tpj[:], in_=grows[c][:], identity=ident[:])
            jTc = jt_pool.tile([128, 128], f32, tag="jt")
            nc.scalar.copy(out=jTc[:], in_=tpj[:])

            tpm = setup_ps.tile([128, 128], f32, tag="sps")
            nc.tensor.transpose(out=tpm[:], in_=mrows[c][:], identity=ident[:])
            mTc = mt_pool.tile([128, 128], f32, tag="mt")
            nc.vector.tensor_copy(out=mTc[:], in_=tpm[:])

            for sub in range(2):
                b = 2 * c + sub
                # ---- projections into JV[b] ----
                pj = setup_ps.tile([KJV, 128], f32, tag="sps")
                nc.tensor.matmul(out=pj[0:N_JOBS, :],
                                 lhsT=jTc[:, sub * PB: sub * PB + N_JOBS],
                                 rhs=wj_s[:], start=True, stop=True)
                nc.tensor.matmul(out=pj[R_V0:R_V0 + N_MA, :],
                                 lhsT=mTc[:, sub * N_MA: (sub + 1) * N_MA],
                                 rhs=wm_s[:], start=True, stop=True)
                nc.scalar.copy(out=jv[b][0:N_JOBS, :], in_=pj[0:N_JOBS, :])
                nc.vector.tensor_copy(out=jv[b][R_V0:R_V0 + N_MA, :],
                                      in_=pj[R_V0:R_V0 + N_MA, :])
                nc.scalar.dma_start(out=jv[b][R_ZD:R_B1 + 1, :], in_=zdb1[:])

                # ---- main pipeline for batch b ----
                lgp = lg_ps.tile([128, 512], f32, tag="lg")
                for ci, (c0, cw) in enumerate(COLCH):
                    h1p = h1_ps.tile([128, 512], f32, tag="h1p")
                    nc.tensor.matmul(out=h1p[:, :cw], lhsT=jv[b][0:KJV, :],
                                     rhs=smat_s[:, c0:c0 + cw],
                                     start=True, stop=True)
                    A = a_pool.tile([128, 512], MMDT, tag="A")
                    if ci % 2 == 0:
                        nc.vector.tensor_scalar_max(out=A[:, :cw], in0=h1p[:, :cw],
                                                    scalar1=0.0)
                    else:
                        nc.scalar.activation(out=A[:, :cw], in_=h1p[:, :cw],
                                             func=Relu)
                    h2p = h2_ps.tile([128, 512], f32, tag="h2p")
                    nc.tensor.matmul(out=h2p[:, :cw], lhsT=w2_s[:], rhs=A[:, :cw],
                                     start=True, stop=True)
                    H2 = h2s_pool.tile([128, 512], MMDT, tag="H2")
                    if ci % 2 == 0:
                        nc.scalar.activation(out=H2[:, :cw], in_=h2p[:, :cw],
                                             func=Relu, bias=b2_s[:, 0:1])
                    else:
                        nc.vector.tensor_scalar(out=H2[:, :cw], in0=h2p[:, :cw],
                                                scalar1=b2_s[:, 0:1], scalar2=0.0,
                                                op0=mybir.AluOpType.add,
                                                op1=mybir.AluOpType.max)
                    # logits chunk -> psum partition 32*ci
                    nc.tensor.matmul(out=lgp[32 * ci:32 * ci + 1, :cw],
                                     lhsT=w3_s[:], rhs=H2[:, :cw],
                                     start=True, stop=True,
                                     tile_position=(0, 32 * ci))
                # one wide copy (only rows 0/32/64/96 carry data)
                stg = st_pool.tile([128, 512], f32, tag="st")
                if b % 2 == 0:
                    nc.scalar.copy(out=stg[0:97, :], in_=lgp[0:97, :])
                else:
                    nc.vector.tensor_copy(out=stg[0:97, :], in_=lgp[0:97, :])
                stg4 = stg[:].rearrange("(a b) f -> a b f", b=32)[:, 0:1, :]
                nc.scalar.dma_start(
                    out=out[b:b + 1, :].rearrange("o (a f) -> o a f", a=4),
                    in_=stg4)

    nc.finalize()
    return nc


_CACHE: dict = {}


def _get_module() -> bass.Bass:
    if "nc" not in _CACHE:
        _CACHE["nc"] = _build_module()
    return _CACHE["nc"]


def _make_in_maps(inputs):
    import ml_dtypes
    np_mm = ml_dtypes.bfloat16 if MMDT == bf16 else np.float32

    ops_emb = np.ascontiguousarray(np.asarray(inputs["ops_emb"], dtype=np.float32))
    ma_emb = np.ascontiguousarray(np.asarray(inputs["ma_emb"], dtype=np.float32))
    next_op = np.asarray(inputs["next_op"])
    dummy = np.asarray(inputs["dummy"], dtype=np.float32)
    W1 = np.ascontiguousarray(np.asarray(inputs["W1"], dtype=np.float32))
    b1 = np.asarray(inputs["b1"], dtype=np.float32).astype(np_mm)
    W2 = np.ascontiguousarray(np.asarray(inputs["W2"], dtype=np.float32).astype(np_mm))
    b2 = np.ascontiguousarray(np.asarray(inputs["b2"], dtype=np.float32))
    W3 = np.ascontiguousarray(np.asarray(inputs["W3"], dtype=np.float32).astype(np_mm))
    smat = _build_smat().astype(np_mm)

    in_maps = []
    for core in range(NCORES):
        bsl = slice(core * BPC, (core + 1) * BPC)
        no = np.asarray(next_op[bsl], dtype=np.int64)          # [BPC, 50]
        gidx = np.zeros((BPC, PB), np.int64)
        gidx[:, :N_JOBS] = no + (np.arange(BPC, dtype=np.int64)[:, None] * N_OPS)
        idx2d = np.ascontiguousarray(
            gidx.reshape(NCHUNK, 128).T.astype(np.int32))      # [128, NCHUNK]
        in_maps.append({
            "ops": ops_emb[bsl].reshape(BPC * N_OPS, E),
            "ma": ma_emb[bsl].reshape(BPC * N_MA, E),
            "idx": idx2d,
            "smat": smat,
            "w1": W1, "w2": W2, "w3": W3,
            "b1v": b1, "b2v": b2, "dvec": dummy,
        })
    return in_maps


def _run(inputs, trace=False, **kw):
    action_mask = np.asarray(inputs["action_mask"])
    b3 = np.asarray(inputs["b3"], dtype=np.float32)
    nc = _get_module()
    in_maps = _make_in_maps(inputs)
    res = run_bass_kernel_spmd(nc, in_maps, core_ids=list(range(NCORES)),
                               trace=trace, **kw)
    logits = np.concatenate([r["out"][:, :NPAIR] for r in res.results], axis=0)
    logits = (logits + b3.reshape(-1)[0]).astype(np.float32)
    return (logits, action_mask), res


def kernel(**inputs):
    out, _ = _run(inputs)
    return out
